# revision 29
# baseline (speedup 1.0000x reference)
"""Bass/Trainium2 kernel for nn_Attention_10299331576042.

Math: reference computes
    energies = enc @ W.T + b          # [S, H]
    scores   = energies @ hidden      # [S]
    attn     = softmax(scores)        # [1, 1, S]

Algebra: scores = enc @ (hidden @ W) + (b . hidden).  The (b . hidden) term is
a constant shift across the sequence axis and softmax is shift-invariant, so it
drops out exactly.  The problem reduces to the memory-bound matvec
    v = hidden @ W                    # [H]
    scores = enc @ v                  # [S]
followed by a softmax over S = 32768 scores.

Numerics: inputs are downcast to fp16 host-side (half the HBM traffic; the
dominant cost is streaming enc).  Products are exact in fp32 (fp16*fp16 fits)
and all accumulation is fp32 (PSUM / ACT accumulator), so the only error is
the input quantization: measured attn rel-err ~5e-3 against the fp32
reference, well inside the 2e-2 gate (the softmax here is sharp, score sigma
~35, which makes it forgiving of small score noise).

Layout: enc shards are transposed host-side to [H, SS] so the matvec runs on
the TensorEngine with H on partitions: for each h-chunk c and output column j,
  matmul(psum_s[:, j], lhsT=encT[:, c, j::32], rhs=v[:, c])
accumulates psum_s[p, j] = scores[p*32 + j] directly in the [128, 32] layout
the softmax tail wants.  v itself comes from 64 more PE matmuls against the
replicated W (also fp16).

Launch 1 (8 cores, sequence-parallel): 3 load DMAs (hidden, W, encT) on one
HWDGE ring, 320 PE matmuls, then a per-partition softmax prepass: nm =
-max_j(s), e = exp(s + nm) (ACT, fp32 accum z), all packed into ONE [128, 34]
fp16 output DMA (e | nm | z).

Launch 2 (8 cores): each core loads the per-(core, partition) stats of ALL
cores (rolled so its own column is first) plus its own e shard, computes the
global max via a Pool-engine cross-partition min of nm, t = exp(m - M) on ACT,
Z = sum(t*z) via a second Pool cross-partition reduce, and rescales its e
shard with one two-scalar DVE op: attn = (e * t0) * (1/Z).  Output is the
contiguous fp32 attn shard.

Walrus constraints honoured (found by a previous session): at most ONE sync
wait per instruction (absorber ops make later deps transitive through vector
clocks), no InstISA ops, split kernel-tail drain.
"""

from contextlib import ExitStack

import numpy as np

import concourse.bass as bass
import concourse.tile as tile
from concourse import mybir
from concourse.bass_utils import run_bass_kernel_spmd
from concourse.vector_clock import ScopedClock


class _SplitDrainTileContext(tile.TileContext):
    """TileContext whose kernel-tail drain is split into single-wait drains.

    The walrus build in this container rejects any instruction carrying more
    than one sync wait; the stock tail drain waits on every semaphore at once.
    A chain of drains, each waiting on one semaphore, is semantically
    identical (all waits complete before the end-of-kernel barrier).
    """

    def _drain_and_barrier(self, tick_clock, wait_clock):
        drain_inst = self.nc.sync.drain()
        wait_clock.add_sem_waits(
            drain_inst.ins, ScopedClock({None: tick_clock.global_clock})
        )
        si = drain_inst.ins.sync_info
        waits = list(si.on_wait) if si is not None and si.on_wait else []
        if len(waits) > 1:
            drain_inst.ins.sync_info = mybir.SyncInfo(
                on_wait=[waits[0]],
                on_update=list(si.on_update) if si.on_update else [],
            )
            for w in waits[1:]:
                extra = self.nc.sync.drain().ins
                extra.sync_info = mybir.SyncInfo(on_wait=[w], on_update=[])

        self.nc.all_engine_barrier()
        assert self.sems is not None
        popped = self.nc._tile_sem_poison_stack.pop()
        assert popped is self._sem_poison
        self.nc.clear_and_free_semaphores(list(self.sems.allocated().values()))
        self.nc.all_engine_barrier()

N_CORES = 8
S = 32768
H = 1024
SS = S // N_CORES          # 4096 rows per core
P = 128                    # partitions
NCH = H // P               # 8 h-chunks
JW = SS // P               # 32 score columns per partition
F32 = mybir.dt.float32
F16 = mybir.dt.float16

TRACE = False
LAST_PERF = {}

_NC_CACHE = {}


def _build_scores_nc():
    """Launch 1: e/nm/z prepass for one 4096-row enc shard (all-fp16 loads)."""
    nc = bass.Bass("TRN2", target_bir_lowering=False, debug=False)
    # encT: host-transposed shard, [H, SS] fp16 row-major
    encT = nc.dram_tensor("encT", [H, SS], F16, kind="ExternalInput").ap()
    # wh: W row-major with hidden packed per row: wh[d, 0:H] = W[d],
    # wh[d, H] = hidden[d] (one fewer DMA)
    wh = nc.dram_tensor("wh", [H, H + 2], F16, kind="ExternalInput").ap()
    # eo packs e[128,32] | nm[128,1] | z[128,1], all fp16
    eo = nc.dram_tensor("eo", [P * 34], F16, kind="ExternalOutput").ap()

    with _SplitDrainTileContext(nc) as tc, ExitStack() as ctx:
        pool = ctx.enter_context(tc.tile_pool(name="p", bufs=1))
        psum = ctx.enter_context(tc.tile_pool(name="ps", bufs=1, space="PSUM"))

        # ---- loads: zero-wait DMAs on the SP ring.  W and hidden are packed
        # host-side into one [8, 128, 1026] fp16 buffer (row = W row | hidden
        # elem) so they arrive in a single DMA.  enc is split (2,2,2,1,1)
        # chunks: 6 loads + 1 store = 7 HWDGE DMAs total (< 8 sems, no
        # recycling waits) and only one chunk's matmuls remain after the
        # last byte lands.
        wh3 = pool.tile([P, NCH, H + 2], F16)
        nc.sync.dma_start(out=wh3, in_=wh.rearrange("(c p) h -> p c h", p=P))
        w3 = wh3
        enc6 = encT.rearrange("(c p) (m j) -> p c m j", p=P, j=JW)
        enc4 = []
        for c0, cn in ((0, 2), (2, 2), (4, 2), (6, 1), (7, 1)):
            t = pool.tile([P, cn, P, JW], F16, name=f"enc{c0}")
            nc.sync.dma_start(out=t, in_=enc6[:, c0:c0 + cn])
            for i in range(cn):
                enc4.append((t, i))

        # ---- v[c*128+q] = sum_d hidden[d] W[d, c*128+q], PE-accumulated
        psum_v = psum.tile([P, NCH], F32, tag="v")
        for c in range(NCH):
            for dc in range(NCH):
                nc.tensor.matmul(
                    psum_v[:, c:c + 1],
                    lhsT=w3[:, dc, c * P:(c + 1) * P],
                    rhs=w3[:, dc, H:H + 1],
                    start=(dc == 0),
                    stop=(dc == NCH - 1),
                )
        # v as fp16 plus an fp16 residual: scores use v16 + dv16, which
        # removes the fp16(v) quantization from the score error entirely.
        v_sb = pool.tile([P, NCH], F16)
        nc.vector.tensor_copy(out=v_sb, in_=psum_v)
        dv_sb = pool.tile([P, NCH], F16)
        nc.vector.tensor_sub(dv_sb, psum_v, v_sb)
        # PE absorber: observe the DVE tick so score matmuls carry only the
        # enc DMA wait.
        ptiny = psum.tile([1, 2], F32, tag="tiny")
        nc.tensor.matmul(
            ptiny[:, 0:1], lhsT=dv_sb[0:1, 0:1], rhs=dv_sb[0:1, 0:1],
            start=True, stop=True,
        )

        # ---- scores: psum_parts[p, slot, j] = partial score.  Each (slot, j)
        # is one CONTIGUOUS accumulation group (interleaved start/stop groups
        # in a bank accumulate incorrectly), slot granularity follows the enc
        # DMA grouping so c-outer order overlaps the stream and only chunk
        # 7's matmuls remain after the last byte.
        slots = ((0, 2), (2, 2), (4, 2), (6, 1), (7, 1))
        psum_parts = psum.tile([P, len(slots), JW], F32, tag="s")
        for si, (c0, cn) in enumerate(slots):
            for j in range(JW):
                n = 2 * cn
                k = 0
                for c in range(c0, c0 + cn):
                    gt, cc = enc4[c]
                    for vv in (v_sb, dv_sb):
                        nc.tensor.matmul(
                            psum_parts[:, si, j:j + 1],
                            lhsT=gt[:, cc, :, j],
                            rhs=vv[:, c:c + 1],
                            start=(k == 0),
                            stop=(k == n - 1),
                        )
                        k += 1
        sc_sb = pool.tile([P, JW], F32)
        parts_T = bass.AP(
            tensor=psum_parts.tensor,
            offset=psum_parts.offset,
            ap=[list(psum_parts.ap[0]), list(psum_parts.ap[2]),
                list(psum_parts.ap[1])],
        )
        nc.vector.tensor_reduce(
            out=sc_sb, in_=parts_T, axis=mybir.AxisListType.X,
            op=mybir.AluOpType.add,
        )

        # ---- softmax prepass: nm16 = -fp16(max_j s) (fp16 so launch 2 sees
        # the exact bias value), e = exp(s + nm16), z = sum_j e accumulated
        # straight into the fp16 output (error ~5e-4, verified on device).
        # Tail ops run on ACT so the out DMA carries a single ACT wait; the
        # m copy doubles as ACT's DVE absorber.
        out34 = pool.tile([P, 34], F16)
        nm16 = pool.tile([P, 1], F16)
        nc.vector.tensor_reduce(
            out=nm16, in_=sc_sb, axis=mybir.AxisListType.X,
            op=mybir.AluOpType.max, negate=True,
        )
        nc.scalar.mul(out34[:, 32:33], nm16, -1.0)  # m16 = +max
        with nc.allow_low_precision(reason="z fp16 accum, rel err ~5e-4"):
            nc.scalar.activation(
                out=out34[:, 0:32], in_=sc_sb,
                func=mybir.ActivationFunctionType.Exp,
                bias=nm16, scale=1.0, accum_out=out34[:, 33:34],
            )
        # store via the idle SWDGE queue (cheaper post-wait prep than HWDGE)
        nc.gpsimd.dma_start(out=eo.rearrange("(p x) -> p x", x=34), in_=out34)
    return nc


def _build_softmax_nc():
    """Launch 2: global combine + rescale of one core's e shard."""
    nc = bass.Bass("TRN2", target_bir_lowering=False, debug=False)
    # mze: [128, 48] fp16 = m[128, 8] | z[128, 8] | e[128, 32]; m/z columns
    # are rolled so column 0 = own core (one load DMA for everything)
    mze = nc.dram_tensor("mze", [P * 48], F16, kind="ExternalInput").ap()
    attn = nc.dram_tensor("attn", [SS], F32, kind="ExternalOutput").ap()

    with _SplitDrainTileContext(nc) as tc, ExitStack() as ctx:
        pool = ctx.enter_context(tc.tile_pool(name="p", bufs=1))
        psum = ctx.enter_context(tc.tile_pool(name="ps", bufs=1, space="PSUM"))

        nones16 = pool.tile([1, P], F16)   # -1s: rank-1 bcast of M yields -M
        nc.vector.memset(nones16, -1.0)
        ones32 = pool.tile([1, P], F32)
        nc.vector.memset(ones32, 1.0)

        mze_sb = pool.tile([P, 48], F16)
        nc.sync.dma_start(out=mze_sb, in_=mze.rearrange("(p x) -> p x", x=48))
        nmz_sb = mze_sb[:, 0:16]
        e3 = mze_sb[:, 16:48]
        # DVE + ACT absorbers for the load (exp then runs with no waits)
        junk_e = pool.tile([P, 2], F16)
        nc.vector.tensor_copy(out=junk_e, in_=e3[:, 0:2])
        junk_a = pool.tile([P, 2], F16)
        nc.scalar.copy(out=junk_a, in_=nmz_sb[:, 0:2])

        # M = max over all (p, k) of m  (Pool cross-partition reduce)
        mg = pool.tile([1, 1], F16)
        nc.gpsimd.tensor_reduce(
            out=mg, in_=nmz_sb[:, 0:NCH], axis=mybir.AxisListType.XYZWC,
            op=mybir.AluOpType.max,
        )
        # PE absorber for the -1s memset, then broadcast -M to all partitions
        # via rank-1 matmul against the -1s vector.
        ptiny = psum.tile([1, 2], F32, tag="tiny")
        nc.tensor.matmul(
            ptiny[:, 0:1], lhsT=nones16[:, 0:1], rhs=nones16[:, 0:1],
            start=True, stop=True,
        )
        negm_ps = psum.tile([P, 1], F32, tag="negm")
        nc.tensor.matmul(negm_ps, lhsT=nones16, rhs=mg, start=True, stop=True)
        # ACT-local copy: the exp that follows then carries no waits at all
        negm_sb = pool.tile([P, 1], F32)
        nc.scalar.copy(out=negm_sb, in_=negm_ps)

        # t = exp(m - M); column 0 is this core's factor
        t = pool.tile([P, NCH], F32)
        nc.scalar.activation(
            out=t, in_=nmz_sb[:, 0:NCH],
            func=mybir.ActivationFunctionType.Exp,
            bias=negm_sb, scale=1.0,
        )
        # DVE absorber on t so the final mul only waits on PE
        junk_t = pool.tile([P, 2], F32)
        nc.vector.tensor_copy(out=junk_t, in_=t[:, 0:2])
        # Pool-local multiply + cross-partition sum (one fewer engine hop)
        tz = pool.tile([P, NCH], F32)
        nc.gpsimd.tensor_mul(tz, t, nmz_sb[:, NCH:16])
        zsum = pool.tile([1, 1], F32)
        nc.gpsimd.tensor_reduce(
            out=zsum, in_=tz, axis=mybir.AxisListType.XYZWC,
            op=mybir.AluOpType.add,
        )
        rz = pool.tile([1, 1], F32)
        nc.vector.reciprocal(rz, zsum)
        rz_ps = psum.tile([P, 1], F32, tag="rz")
        nc.tensor.matmul(rz_ps, lhsT=ones32, rhs=rz, start=True, stop=True)

        # attn = (e * t[:,0]) * (1/Z)
        attn_sb = pool.tile([P, JW], F32)
        nc.vector.tensor_scalar(
            out=attn_sb, in0=e3, scalar1=t[:, 0:1], scalar2=rz_ps,
            op0=mybir.AluOpType.mult, op1=mybir.AluOpType.mult,
        )
        nc.gpsimd.dma_start(out=attn.rearrange("(p j) -> p j", p=P), in_=attn_sb)
    return nc


def _get_nc(name, builder):
    if name not in _NC_CACHE:
        _NC_CACHE[name] = builder()
    return _NC_CACHE[name]


def kernel(hidden, encoder_outputs, W, b):
    hid16 = np.asarray(hidden, dtype=np.float16)
    enc = np.asarray(encoder_outputs)
    W16 = np.asarray(W, dtype=np.float16)
    # b drops out of softmax (constant shift across seq_len)

    # W and hidden packed into one buffer: wh[d] = W[d, :] | hidden[d] | pad
    wh16 = np.zeros((H, H + 2), dtype=np.float16)
    wh16[:, 0:H] = W16
    wh16[:, H] = hid16

    # Per-core transposed fp16 enc shards: [H, SS] row-major
    encT16 = [
        np.ascontiguousarray(enc[k * SS:(k + 1) * SS].T.astype(np.float16))
        for k in range(N_CORES)
    ]

    nc_scores = _get_nc("scores", _build_scores_nc)
    in_maps = [
        {"encT": encT16[k], "wh": wh16}
        for k in range(N_CORES)
    ]
    res = run_bass_kernel_spmd(
        nc_scores, in_maps, core_ids=list(range(N_CORES)), trace=TRACE
    )
    LAST_PERF["scores"] = res

    eo = [res.results[k]["eo"].reshape(P, 34) for k in range(N_CORES)]
    M = np.stack([eo[k][:, 32] for k in range(N_CORES)], axis=1)  # [128, 8] f16
    Z = np.stack([eo[k][:, 33] for k in range(N_CORES)], axis=1)  # [128, 8] f16

    nc_soft = _get_nc("softmax", _build_softmax_nc)
    in_maps2 = [
        {
            "mze": np.ascontiguousarray(
                np.concatenate(
                    [np.roll(M, -k, axis=1), np.roll(Z, -k, axis=1),
                     eo[k][:, 0:32]], axis=1
                )
            ).reshape(-1),
        }
        for k in range(N_CORES)
    ]
    res2 = run_bass_kernel_spmd(
        nc_soft, in_maps2, core_ids=list(range(N_CORES)), trace=TRACE
    )
    LAST_PERF["softmax"] = res2

    attn = np.concatenate([res2.results[k]["attn"] for k in range(N_CORES)])
    return np.asarray(attn, dtype=np.float32).reshape(1, 1, S)


# revision 33
# speedup vs baseline: 1.0241x; 1.0241x over previous
"""Bass/Trainium2 kernel for nn_Attention_10299331576042.

Math: reference computes
    energies = enc @ W.T + b          # [S, H]
    scores   = energies @ hidden      # [S]
    attn     = softmax(scores)        # [1, 1, S]

Algebra: scores = enc @ (hidden @ W) + (b . hidden).  The (b . hidden) term is
a constant shift across the sequence axis and softmax is shift-invariant, so it
drops out exactly.  The problem reduces to the memory-bound matvec
    v = hidden @ W                    # [H]
    scores = enc @ v                  # [S]
followed by a softmax over S = 32768 scores.

Numerics: inputs are downcast to fp16 host-side (half the HBM traffic; the
dominant cost is streaming enc).  Products are exact in fp32 (fp16*fp16 fits)
and all accumulation is fp32 (PSUM / ACT accumulator), so the only error is
the input quantization: measured attn rel-err ~5e-3 against the fp32
reference, well inside the 2e-2 gate (the softmax here is sharp, score sigma
~35, which makes it forgiving of small score noise).

Layout: enc shards are transposed host-side to [H, SS] so the matvec runs on
the TensorEngine with H on partitions: for each h-chunk c and output column j,
  matmul(psum_parts[:, slot, j], lhsT=encT[:, c, j::32], rhs=v[:, c])
lands partial scores for row p*32 + j directly in the [128, 32] layout the
softmax tail wants (out free size is 1, so these 1024+64 matmuls are nearly
free in the cost model; the kernel is purely enc-DMA-bound at 360 B/ns).  v
comes from 64 PE matmuls against the replicated W, and is applied as
fp16(v) + fp16(v - fp16(v)) so its quantization drops out of the scores.

Launch 1 (8 cores, sequence-parallel): 6 zero-wait load DMAs on the SP ring
(W|hidden packed buffer, then enc in (2,2,2,1,1)-chunk groups so only the
last chunk's matmuls trail the stream), 1088 PE matmuls, a DVE partial-sum +
negated-max reduce, one ACT exp with per-partition fp16 bias and fp16 z
accumulation, and ONE packed [128, 34] fp16 output DMA (e | m | z).

Launch 2 (8 cores): each core loads ONE [128, 48] fp16 buffer (m and z stats
of all cores, rolled so its own column is first, plus its own e shard),
computes the global max M with a Pool-engine cross-partition reduce, t =
exp(m - M) on ACT (bias -M from a rank-1 PE broadcast against a -1s vector),
Z = sum(t*z) via Pool multiply + cross-partition reduce, and rescales its e
shard with one two-scalar DVE op: attn = (e * t0) * (1/Z).  Output is the
contiguous fp32 attn shard.

Walrus constraints honoured (found by a previous session): at most ONE sync
wait per instruction (absorber ops make later deps transitive through vector
clocks), no InstISA ops, split kernel-tail drain.
"""

from contextlib import ExitStack

import numpy as np

import concourse.bass as bass
import concourse.tile as tile
from concourse import mybir
from concourse.bass_utils import run_bass_kernel_spmd
from concourse.vector_clock import ScopedClock


class _SplitDrainTileContext(tile.TileContext):
    """TileContext whose kernel-tail drain is split into single-wait drains.

    The walrus build in this container rejects any instruction carrying more
    than one sync wait; the stock tail drain waits on every semaphore at once.
    A chain of drains, each waiting on one semaphore, is semantically
    identical (all waits complete before the end-of-kernel barrier).
    """

    def _drain_and_barrier(self, tick_clock, wait_clock):
        drain_inst = self.nc.sync.drain()
        wait_clock.add_sem_waits(
            drain_inst.ins, ScopedClock({None: tick_clock.global_clock})
        )
        si = drain_inst.ins.sync_info
        waits = list(si.on_wait) if si is not None and si.on_wait else []
        if len(waits) > 1:
            drain_inst.ins.sync_info = mybir.SyncInfo(
                on_wait=[waits[0]],
                on_update=list(si.on_update) if si.on_update else [],
            )
            for w in waits[1:]:
                extra = self.nc.sync.drain().ins
                extra.sync_info = mybir.SyncInfo(on_wait=[w], on_update=[])

        self.nc.all_engine_barrier()
        assert self.sems is not None
        popped = self.nc._tile_sem_poison_stack.pop()
        assert popped is self._sem_poison
        self.nc.clear_and_free_semaphores(list(self.sems.allocated().values()))
        self.nc.all_engine_barrier()

N_CORES = 8
S = 32768
H = 1024
SS = S // N_CORES          # 4096 rows per core
P = 128                    # partitions
NCH = H // P               # 8 h-chunks
JW = SS // P               # 32 score columns per partition
F32 = mybir.dt.float32
F16 = mybir.dt.float16

TRACE = False
LAST_PERF = {}

_NC_CACHE = {}


def _build_scores_nc():
    """Launch 1: e/nm/z prepass for one 4096-row enc shard (all-fp16 loads)."""
    nc = bass.Bass("TRN2", target_bir_lowering=False, debug=False)
    # encT: host-transposed shard, [H, SS] fp16 row-major
    encT = nc.dram_tensor("encT", [H, SS], F16, kind="ExternalInput").ap()
    # wh: W row-major with hidden packed per row: wh[d, 0:H] = W[d],
    # wh[d, H] = hidden[d] (one fewer DMA)
    wh = nc.dram_tensor("wh", [H, H + 2], F16, kind="ExternalInput").ap()
    # eo packs e[128,32] | m[128,1] | z[128,1], all fp16
    eo = nc.dram_tensor("eo", [P * 34], F16, kind="ExternalOutput").ap()

    with _SplitDrainTileContext(nc) as tc, ExitStack() as ctx:
        pool = ctx.enter_context(tc.tile_pool(name="p", bufs=1))
        psum = ctx.enter_context(tc.tile_pool(name="ps", bufs=1, space="PSUM"))

        # ---- loads: zero-wait DMAs on the SP ring.  W and hidden are packed
        # host-side into one [8, 128, 1026] fp16 buffer (row = W row | hidden
        # elem) so they arrive in a single DMA.  enc is split (2,2,2,1,1)
        # chunks: 6 loads + 1 store = 7 HWDGE DMAs total (< 8 sems, no
        # recycling waits) and only one chunk's matmuls remain after the
        # last byte lands.
        wh3 = pool.tile([P, NCH, H + 2], F16)
        nc.sync.dma_start(out=wh3, in_=wh.rearrange("(c p) h -> p c h", p=P))
        w3 = wh3
        enc6 = encT.rearrange("(c p) (m j) -> p c m j", p=P, j=JW)
        enc4 = []
        for c0, cn in ((0, 2), (2, 2), (4, 2), (6, 1), (7, 1)):
            t = pool.tile([P, cn, P, JW], F16, name=f"enc{c0}")
            nc.sync.dma_start(out=t, in_=enc6[:, c0:c0 + cn])
            for i in range(cn):
                enc4.append((t, i))

        # ---- v[c*128+q] = sum_d hidden[d] W[d, c*128+q], PE-accumulated
        psum_v = psum.tile([P, NCH], F32, tag="v")
        for c in range(NCH):
            for dc in range(NCH):
                nc.tensor.matmul(
                    psum_v[:, c:c + 1],
                    lhsT=w3[:, dc, c * P:(c + 1) * P],
                    rhs=w3[:, dc, H:H + 1],
                    start=(dc == 0),
                    stop=(dc == NCH - 1),
                )
        # v as fp16 plus an fp16 residual: scores use v16 + dv16, which
        # removes the fp16(v) quantization from the score error entirely.
        v_sb = pool.tile([P, NCH], F16)
        nc.vector.tensor_copy(out=v_sb, in_=psum_v)
        dv_sb = pool.tile([P, NCH], F16)
        nc.vector.tensor_sub(dv_sb, psum_v, v_sb)
        # PE absorber: observe the DVE tick so score matmuls carry only the
        # enc DMA wait.
        ptiny = psum.tile([1, 2], F32, tag="tiny")
        nc.tensor.matmul(
            ptiny[:, 0:1], lhsT=dv_sb[0:1, 0:1], rhs=dv_sb[0:1, 0:1],
            start=True, stop=True,
        )

        # ---- scores: psum_parts[p, slot, j] = partial score.  Each (slot, j)
        # is one CONTIGUOUS accumulation group (interleaved start/stop groups
        # in a bank accumulate incorrectly), slot granularity follows the enc
        # DMA grouping so c-outer order overlaps the stream and only chunk
        # 7's matmuls remain after the last byte.
        slots = ((0, 2), (2, 2), (4, 2), (6, 1), (7, 1))
        psum_parts = psum.tile([P, len(slots), JW], F32, tag="s")
        for si, (c0, cn) in enumerate(slots):
            for j in range(JW):
                n = 2 * cn
                k = 0
                for c in range(c0, c0 + cn):
                    gt, cc = enc4[c]
                    for vv in (v_sb, dv_sb):
                        nc.tensor.matmul(
                            psum_parts[:, si, j:j + 1],
                            lhsT=gt[:, cc, :, j],
                            rhs=vv[:, c:c + 1],
                            start=(k == 0),
                            stop=(k == n - 1),
                        )
                        k += 1
        sc_sb = pool.tile([P, JW], F32)
        parts_T = bass.AP(
            tensor=psum_parts.tensor,
            offset=psum_parts.offset,
            ap=[list(psum_parts.ap[0]), list(psum_parts.ap[2]),
                list(psum_parts.ap[1])],
        )
        nc.vector.tensor_reduce(
            out=sc_sb, in_=parts_T, axis=mybir.AxisListType.X,
            op=mybir.AluOpType.add,
        )

        # ---- softmax prepass: nm16 = -fp16(max_j s) (fp16 so launch 2 sees
        # the exact bias value), e = exp(s + nm16), z = sum_j e accumulated
        # straight into the fp16 output (error ~5e-4, verified on device).
        # Tail ops run on ACT so the out DMA carries a single ACT wait; the
        # m copy doubles as ACT's DVE absorber.
        out34 = pool.tile([P, 34], F16)
        nm16 = pool.tile([P, 1], F16)
        nc.vector.tensor_reduce(
            out=nm16, in_=sc_sb, axis=mybir.AxisListType.X,
            op=mybir.AluOpType.max, negate=True,
        )
        nc.scalar.mul(out34[:, 32:33], nm16, -1.0)  # m16 = +max
        with nc.allow_low_precision(reason="z fp16 accum, rel err ~5e-4"):
            nc.scalar.activation(
                out=out34[:, 0:32], in_=sc_sb,
                func=mybir.ActivationFunctionType.Exp,
                bias=nm16, scale=1.0, accum_out=out34[:, 33:34],
            )
        nc.sync.dma_start(out=eo.rearrange("(p x) -> p x", x=34), in_=out34)
    return nc


def _build_softmax_nc():
    """Launch 2: global combine + rescale of one core's e shard."""
    nc = bass.Bass("TRN2", target_bir_lowering=False, debug=False)
    # mze: [128, 48] fp16 = m[128, 8] | z[128, 8] | e[128, 32]; m/z columns
    # are rolled so column 0 = own core (one load DMA for everything)
    mze = nc.dram_tensor("mze", [P * 48], F16, kind="ExternalInput").ap()
    attn = nc.dram_tensor("attn", [SS], F32, kind="ExternalOutput").ap()

    with _SplitDrainTileContext(nc) as tc, ExitStack() as ctx:
        pool = ctx.enter_context(tc.tile_pool(name="p", bufs=1))
        psum = ctx.enter_context(tc.tile_pool(name="ps", bufs=1, space="PSUM"))

        nones16 = pool.tile([1, P], F16)   # -1s: rank-1 bcast of M yields -M
        nc.vector.memset(nones16, -1.0)
        ones32 = pool.tile([1, P], F32)
        nc.vector.memset(ones32, 1.0)

        mze_sb = pool.tile([P, 48], F16)
        nc.sync.dma_start(out=mze_sb, in_=mze.rearrange("(p x) -> p x", x=48))
        nmz_sb = mze_sb[:, 0:16]
        e3 = mze_sb[:, 16:48]
        # DVE + ACT absorbers for the load (exp then runs with no waits)
        junk_e = pool.tile([P, 2], F16)
        nc.vector.tensor_copy(out=junk_e, in_=e3[:, 0:2])
        junk_a = pool.tile([P, 2], F16)
        nc.scalar.copy(out=junk_a, in_=nmz_sb[:, 0:2])

        # M = max over all (p, k) of m  (Pool cross-partition reduce)
        mg = pool.tile([1, 1], F16)
        nc.gpsimd.tensor_reduce(
            out=mg, in_=nmz_sb[:, 0:NCH], axis=mybir.AxisListType.XYZWC,
            op=mybir.AluOpType.max,
        )
        # PE absorber for the -1s memset, then broadcast -M to all partitions
        # via rank-1 matmul against the -1s vector.
        ptiny = psum.tile([1, 2], F32, tag="tiny")
        nc.tensor.matmul(
            ptiny[:, 0:1], lhsT=nones16[:, 0:1], rhs=nones16[:, 0:1],
            start=True, stop=True,
        )
        negm_ps = psum.tile([P, 1], F32, tag="negm")
        nc.tensor.matmul(negm_ps, lhsT=nones16, rhs=mg, start=True, stop=True)
        # ACT-local copy: the exp that follows then carries no waits at all
        negm_sb = pool.tile([P, 1], F32)
        nc.scalar.copy(out=negm_sb, in_=negm_ps)

        # t = exp(m - M); column 0 is this core's factor
        t = pool.tile([P, NCH], F32)
        nc.scalar.activation(
            out=t, in_=nmz_sb[:, 0:NCH],
            func=mybir.ActivationFunctionType.Exp,
            bias=negm_sb, scale=1.0,
        )
        # DVE absorber on t so the final mul only waits on PE
        junk_t = pool.tile([P, 2], F32)
        nc.vector.tensor_copy(out=junk_t, in_=t[:, 0:2])
        # Pool-local multiply + cross-partition sum (one fewer engine hop)
        tz = pool.tile([P, NCH], F32)
        nc.gpsimd.tensor_mul(tz, t, nmz_sb[:, NCH:16])
        zsum = pool.tile([1, 1], F32)
        nc.gpsimd.tensor_reduce(
            out=zsum, in_=tz, axis=mybir.AxisListType.XYZWC,
            op=mybir.AluOpType.add,
        )
        rz = pool.tile([1, 1], F32)
        nc.vector.reciprocal(rz, zsum)
        rz_ps = psum.tile([P, 1], F32, tag="rz")
        nc.tensor.matmul(rz_ps, lhsT=ones32, rhs=rz, start=True, stop=True)

        # attn = (e * t[:,0]) * (1/Z)
        attn_sb = pool.tile([P, JW], F32)
        nc.vector.tensor_scalar(
            out=attn_sb, in0=e3, scalar1=t[:, 0:1], scalar2=rz_ps,
            op0=mybir.AluOpType.mult, op1=mybir.AluOpType.mult,
        )
        nc.sync.dma_start(out=attn.rearrange("(p j) -> p j", p=P), in_=attn_sb)
    return nc


def _get_nc(name, builder):
    if name not in _NC_CACHE:
        _NC_CACHE[name] = builder()
    return _NC_CACHE[name]


def kernel(hidden, encoder_outputs, W, b):
    hid16 = np.asarray(hidden, dtype=np.float16)
    enc = np.asarray(encoder_outputs)
    W16 = np.asarray(W, dtype=np.float16)
    # b drops out of softmax (constant shift across seq_len)

    # W and hidden packed into one buffer: wh[d] = W[d, :] | hidden[d] | pad
    wh16 = np.zeros((H, H + 2), dtype=np.float16)
    wh16[:, 0:H] = W16
    wh16[:, H] = hid16

    # Per-core transposed fp16 enc shards: [H, SS] row-major
    encT16 = [
        np.ascontiguousarray(enc[k * SS:(k + 1) * SS].T.astype(np.float16))
        for k in range(N_CORES)
    ]

    nc_scores = _get_nc("scores", _build_scores_nc)
    in_maps = [
        {"encT": encT16[k], "wh": wh16}
        for k in range(N_CORES)
    ]
    res = run_bass_kernel_spmd(
        nc_scores, in_maps, core_ids=list(range(N_CORES)), trace=TRACE
    )
    LAST_PERF["scores"] = res

    eo = [res.results[k]["eo"].reshape(P, 34) for k in range(N_CORES)]
    M = np.stack([eo[k][:, 32] for k in range(N_CORES)], axis=1)  # [128, 8] f16
    Z = np.stack([eo[k][:, 33] for k in range(N_CORES)], axis=1)  # [128, 8] f16

    nc_soft = _get_nc("softmax", _build_softmax_nc)
    in_maps2 = [
        {
            "mze": np.ascontiguousarray(
                np.concatenate(
                    [np.roll(M, -k, axis=1), np.roll(Z, -k, axis=1),
                     eo[k][:, 0:32]], axis=1
                )
            ).reshape(-1),
        }
        for k in range(N_CORES)
    ]
    res2 = run_bass_kernel_spmd(
        nc_soft, in_maps2, core_ids=list(range(N_CORES)), trace=TRACE
    )
    LAST_PERF["softmax"] = res2

    attn = np.concatenate([res2.results[k]["attn"] for k in range(N_CORES)])
    return np.asarray(attn, dtype=np.float32).reshape(1, 1, S)
